# revision 16
# baseline (speedup 1.0000x reference)
"""Distributed multi-head attention kernel for one TRN2 chip (8 NeuronCores).

Problem: x[2, 2048, 1024] -> fused QKV proj (16 heads x 64) -> softmax attention
-> output proj, weights packed as in the reference (qkv interleaved [3, h, d]).

Sharding: 2-way data parallel on batch x 4-way tensor parallel on heads.
Core c = (b = c // 4, g = c % 4) gets batch b and heads [4g, 4g+4).
W_qkv column-sharded by head, W_out row-sharded; per-block bf16
ReduceScatter(add) over each batch group of 4 cores combines the partial
output projections; core (b, g) returns 128-row slices of batch b's output.

v2 pipeline (vs v1 baseline at ~377us):
  - fine-grained per-rb tiles (xT/kT/qT) so attention block 0 starts as
    soon as the first 512-row block's projections land (software pipeline
    of input load/proj with block-0 attention).
  - ScalarE runs exp only (the attention bottleneck); DVE does all psum
    evacuation and casts; normalization reads PV psum rows directly.
  - per-block ReduceScatter emitted as soon as the block's outproj is done.
"""
import numpy as np

from concourse import mybir, tile, bacc
from concourse.bass_utils import run_bass_kernel_spmd

S = 2048       # sequence length (one batch element per core)
D = 1024       # embed dim
HL = 4         # local heads per core
HD = 64        # head dim
QKVC = 3 * HL * HD   # 768 local qkv columns
VOFF = 2 * HL * HD   # 512: V column offset within the shard
BLK = 512      # s_q / s_k block size
NBLK = S // BLK      # 4
KC = S // 128        # 16 s_k chunks
DC = D // 128        # 8 dmodel chunks
F32 = mybir.dt.float32
BF16 = mybir.dt.bfloat16
EXP = mybir.ActivationFunctionType.Exp
SCALE = 1.0 / np.sqrt(HD)

REPLICA_GROUPS = [[0, 1, 2, 3], [4, 5, 6, 7]]


def build_nc():
    from contextlib import ExitStack

    nc = bacc.Bacc("TRN2", target_bir_lowering=False, debug=False, num_devices=8)
    x_ext = nc.declare_dram_parameter("x", [S, D], F32, isOutput=False)
    wqkv_ext = nc.declare_dram_parameter("wqkv", [D, QKVC], F32, isOutput=False)
    bqkv_ext = nc.declare_dram_parameter("bqkv", [QKVC], F32, isOutput=False)
    wout_ext = nc.declare_dram_parameter("wout", [HL * HD, D], F32, isOutput=False)
    bout_ext = nc.declare_dram_parameter("bout", [D], F32, isOutput=False)
    out_ext = nc.declare_dram_parameter("out", [NBLK * 128, D], F32, isOutput=True)

    with tile.TileContext(nc) as tc, ExitStack() as top:
        # ---- persistent pools ----
        const = top.enter_context(tc.tile_pool(name="const", bufs=1))
        xT_pool = top.enter_context(tc.tile_pool(name="xT", bufs=4 * DC))
        kT_pool = top.enter_context(tc.tile_pool(name="kT", bufs=8))
        qT_pool = top.enter_context(tc.tile_pool(name="qT", bufs=8))
        v_pool = top.enter_context(tc.tile_pool(name="v", bufs=KC))
        wq_pool = top.enter_context(tc.tile_pool(name="wq", bufs=DC))
        wqs_pool = top.enter_context(tc.tile_pool(name="wqs", bufs=2))
        woutp = top.enter_context(tc.tile_pool(name="woutp", bufs=2))
        e_pool = top.enter_context(tc.tile_pool(name="e", bufs=16))
        oT_pool = top.enter_context(tc.tile_pool(name="oT", bufs=4))
        pvf_pool = top.enter_context(tc.tile_pool(name="pvf", bufs=3))
        r_pool = top.enter_context(tc.tile_pool(name="recip", bufs=4))
        rb_pool = top.enter_context(tc.tile_pool(name="rbc", bufs=2))
        stage = top.enter_context(tc.tile_pool(name="stage", bufs=4))
        xstage = top.enter_context(tc.tile_pool(name="xstage", bufs=4))
        ostage = top.enter_context(tc.tile_pool(name="ostage", bufs=2))
        rs_dram = top.enter_context(tc.tile_pool(name="rs_dram", bufs=6, space="DRAM"))

        sp_ps = top.enter_context(tc.tile_pool(name="sp_ps", bufs=2, space="PSUM"))
        pv_ps = top.enter_context(tc.tile_pool(name="pv_ps", bufs=2, space="PSUM"))
        proj_ps = top.enter_context(tc.tile_pool(name="proj_ps", bufs=2, space="PSUM"))

        WARMUP = 0
        if WARMUP:
            # ---- ACT exp-table warm-up (table load ~2.7us hides under DMA)
            warm = const.tile([1, 8], F32)
            nc.vector.memset(warm[:, :], 0.25)
            warm2 = const.tile([1, 8], F32)
            nc.scalar.activation(warm2[:, :], warm[:, :], EXP)

            # ---- PE warm-up: dummy matmuls so HAM reaches K=8/8 early
            dum_w = const.tile([128, 128], BF16)
            nc.vector.memset(dum_w[:, :], 0.0)
            dum_x = const.tile([128, 512], BF16)
            nc.vector.memset(dum_x[:, :], 0.0)
            dum_ps = proj_ps.tile([128, 512], F32, tag="proj")
            for i in range(20):
                nc.tensor.matmul(dum_ps[:, :], dum_w[:, :], dum_x[:, :],
                                 start=True, stop=True, skip_group_check=True)
            dum_out = const.tile([1, 8], F32)
            nc.vector.tensor_copy(dum_out[:, :], dum_ps[0:1, 0:8])

        # ---- constants / weights ----
        bqk_sb = const.tile([128, 4], F32)        # per-partition qk bias, col m
        for m in range(4):
            nc.gpsimd.dma_start(out=bqk_sb[:, m:m + 1],
                                in_=bqkv_ext[m * 128:(m + 1) * 128][:, None])
        bv_sb = const.tile([128, HL * HD], F32)   # v bias broadcast across partitions
        nc.gpsimd.dma_start(out=bv_sb[:, :],
                            in_=bqkv_ext[VOFF:QKVC][None, :].to_broadcast((128, HL * HD)))
        bout_full = const.tile([128, D], F32)
        nc.gpsimd.dma_start(out=bout_full[:, :],
                            in_=bout_ext[None, :].to_broadcast((128, D)))

        wout_bf = []
        for p in range(2):
            wf = wqs_pool.tile([128, D], F32, tag="wstage")
            nc.gpsimd.dma_start(out=wf[:, :], in_=wout_ext[p * 128:(p + 1) * 128, :])
            wb = woutp.tile([128, D], BF16, tag="wout_bf")
            nc.vector.tensor_copy(wb[:, :], wf[:, :])
            wout_bf.append(wb)

        # W_qkv loaded per c-chunk in three column groups, K columns first so
        # k_proj (the gate for the first attention chunks) unblocks earliest.
        wq_bf = [wq_pool.tile([128, QKVC], BF16, tag="wq_bf", name="wq_bf")
                 for _ in range(DC)]
        for lo, hi in ((256, 512), (0, 256), (512, 768)):
            for c in range(DC):
                wf = wqs_pool.tile([128, 256], F32, tag="wstage")
                nc.gpsimd.dma_start(
                    out=wf[:, :], in_=wqkv_ext[c * 128:(c + 1) * 128, lo:hi])
                nc.vector.tensor_copy(wq_bf[c][:, lo:hi], wf[:, :])

        # ---- x -> f32 load (HWDGE) -> DVE cast bf16 -> PE transposes ----
        # (X-bar DMA transpose and SWDGE cast-DMAs both produced flaky HW
        # hangs/corruption on this setup; this is the baseline-proven path.)
        from concourse.masks import make_identity
        ident = const.tile([128, 128], BF16)
        make_identity(nc, ident[:, :])

        xb_sb = []
        for sc in range(KC):
            xf = xstage.tile([128, D], F32, tag="xf")
            (nc.sync if sc % 2 == 0 else nc.scalar).dma_start(
                out=xf[:, :], in_=x_ext[sc * 128:(sc + 1) * 128, :])
            xb = xstage.tile([128, D], BF16, tag="xb")
            nc.vector.tensor_copy(xb[:, :], xf[:, :])
            xb_sb.append(xb)

        # xT[c][rb]: [128 dmodel, 512 seq] bf16, written in 4 column pieces
        xT = [[None] * NBLK for _ in range(DC)]
        for rb in range(NBLK):
            for c in range(DC):
                xT[c][rb] = xT_pool.tile([128, BLK], BF16, tag="xT", name="xT")

        def transpose_chunk(sc):
            rb, j = sc // 4, sc % 4
            for c in range(DC):
                tp = proj_ps.tile([128, 128], BF16, tag="proj")
                nc.tensor.transpose(tp[:, :], xb_sb[sc][:, c * 128:(c + 1) * 128],
                                    ident[:, :])
                nc.vector.tensor_copy(xT[c][rb][:, j * 128:(j + 1) * 128], tp[:, :])

        kT = [[None] * NBLK for _ in range(2)]   # [p][rb] -> [128, 512]
        qT = [[None] * NBLK for _ in range(2)]   # [p][blk] -> [128, 512]
        v_sb = [None] * KC                       # [sc] -> [128, 260]

        def qkv_mm(m, rb, tag="proj"):
            ps = proj_ps.tile([128, BLK], F32, tag=tag)
            for c in range(DC):
                nc.tensor.matmul(ps[:, :], wq_bf[c][:, m * 128:(m + 1) * 128],
                                 xT[c][rb][:, :],
                                 start=(c == 0), stop=(c == DC - 1))
            return ps

        def k_proj(mk, rb):
            ps = qkv_mm(2 + mk, rb)
            t = kT_pool.tile([128, BLK], BF16, tag="kT")
            nc.vector.tensor_add(t[:, :], ps[:, :],
                                 bqk_sb[:, 2 + mk:3 + mk].to_broadcast((128, BLK)))
            kT[mk][rb] = t

        def q_proj(mq, blk):
            ps = qkv_mm(mq, blk)
            t = qT_pool.tile([128, BLK], BF16, tag="qT")
            nc.vector.tensor_add(t[:, :], ps[:, :],
                                 bqk_sb[:, mq:mq + 1].to_broadcast((128, BLK)))
            qT[mq][blk] = t

        def v_proj(sc):
            ps = proj_ps.tile([128, HL * HD], F32, tag="proj")
            rb, j = sc // 4, sc % 4
            for c in range(DC):
                nc.tensor.matmul(ps[:, :], xT[c][rb][:, j * 128:(j + 1) * 128],
                                 wq_bf[c][:, VOFF:QKVC],
                                 start=(c == 0), stop=(c == DC - 1))
            t = v_pool.tile([128, HL * (HD + 1)], BF16, tag="v_sb")
            vv = t[:, :].rearrange("p (h n) -> p h n", n=HD + 1)
            nc.vector.memset(vv[:, :, HD:HD + 1], 1.0)
            nc.vector.tensor_add(vv[:, :, 0:HD],
                                 ps[:, :].rearrange("p (h d) -> p h d", d=HD),
                                 bv_sb[:, :].rearrange("p (h d) -> p h d", d=HD))
            v_sb[sc] = t

        def proj_block(rb):
            k_proj(0, rb)
            k_proj(1, rb)
            for j in range(4):
                v_proj(rb * 4 + j)
            q_proj(0, rb)
            q_proj(1, rb)

        # ---- attention pieces ----
        pv_state = {}   # p -> (pvA, pvB)

        def attn_score(blk, p, kc):
            rb, j = kc // 4, kc % 4
            sp = sp_ps.tile([128, 2 * BLK], F32, tag="sp")
            ks = slice(j * 128, (j + 1) * 128)
            nc.tensor.matmul(sp[:, 0:BLK], kT[p][rb][0:64, ks], qT[p][blk][0:64, :],
                             start=True, stop=True)
            nc.tensor.matmul(sp[:, BLK:], kT[p][rb][64:128, ks], qT[p][blk][64:128, :],
                             start=True, stop=True)
            e = e_pool.tile([128, 2 * BLK], BF16, tag="e")
            nc.scalar.activation(e[:, :], sp[:, :], EXP, scale=float(SCALE))
            return e

        def attn_pv(p, kc, e):
            if kc == 0:
                pvA = pv_ps.tile([HD + 1, BLK], F32, tag="pv")
                pvB = pv_ps.tile([HD + 1, BLK], F32, tag="pv")
                pv_state[p] = (pvA, pvB)
            pvA, pvB = pv_state[p]
            nc.tensor.matmul(pvA[:, :],
                             v_sb[kc][:, (2 * p) * (HD + 1):(2 * p + 1) * (HD + 1)],
                             e[:, 0:BLK], start=(kc == 0), stop=(kc == KC - 1),
                             skip_group_check=True)
            nc.tensor.matmul(pvB[:, :],
                             v_sb[kc][:, (2 * p + 1) * (HD + 1):(2 * p + 2) * (HD + 1)],
                             e[:, BLK:], start=(kc == 0), stop=(kc == KC - 1),
                             skip_group_check=True)

        def attn_chunk(blk, p, kc):
            attn_pv(p, kc, attn_score(blk, p, kc))

        def normalize(blk, p):
            # evacuate PV psum (incl. sums row) on DVE, reciprocal, broadcast, mul
            pvA, pvB = pv_state.pop(p)
            ot = oT_pool.tile([128, BLK], BF16, tag="ot")
            for hh, pv in ((0, pvA), (1, pvB)):
                pvf = pvf_pool.tile([HD + 1, BLK], F32, tag="pvf")
                nc.vector.tensor_copy(pvf[:, :], pv[:, :])
                rc = r_pool.tile([1, BLK], F32, tag="rc")
                nc.vector.reciprocal(rc[:, :], pvf[HD:HD + 1, :])
                rbt = rb_pool.tile([64, BLK], F32, tag="rb")
                nc.gpsimd.partition_broadcast(rbt[:, :], rc[:, :])
                nc.vector.tensor_mul(ot[hh * 64:(hh + 1) * 64, :],
                                     pvf[0:HD, :], rbt[:, :])
            return ot

        def outproj_sq(oTb, sq, rs_in):
            st = stage.tile([128, D], BF16, tag="st")
            for nh in range(2):
                po = proj_ps.tile([128, BLK], F32, tag="proj")
                ns = slice(nh * 512, (nh + 1) * 512)
                nc.tensor.matmul(po[:, :], oTb[0][:, sq * 128:(sq + 1) * 128],
                                 wout_bf[0][:, ns], start=True, stop=False)
                nc.tensor.matmul(po[:, :], oTb[1][:, sq * 128:(sq + 1) * 128],
                                 wout_bf[1][:, ns], start=False, stop=True)
                nc.vector.tensor_copy(st[:, ns], po[:, :])
            nc.gpsimd.dma_start(out=rs_in[sq * 128:(sq + 1) * 128, :], in_=st[:, :])

        def emit_rs(blk, rs_in):
            rs_out = rs_dram.tile([128, D], BF16, tag="rs_out")
            nc.gpsimd.collective_compute(
                "ReduceScatter", mybir.AluOpType.add,
                replica_groups=REPLICA_GROUPS,
                ins=[rs_in[:, :].opt()], outs=[rs_out[:, :].opt()])
            ro = ostage.tile([128, D], BF16, tag="ro")
            nc.scalar.dma_start(out=ro[:, :], in_=rs_out[:, :])
            rof = ostage.tile([128, D], F32, tag="rof")
            nc.vector.tensor_add(rof[:, :], ro[:, :], bout_full[:, :])
            nc.scalar.dma_start(out=out_ext[blk * 128:(blk + 1) * 128, :],
                              in_=rof[:, :])

        # ---- fused schedule ----
        # phase A per rb interleaved with block-0 p=0 attention chunks (PSUM
        # has room for exactly one PV pair); p=1 runs inline afterwards.
        for rb in range(NBLK):
            for j in range(4):
                transpose_chunk(rb * 4 + j)
            proj_block(rb)
            for j in range(4):
                attn_chunk(0, 0, rb * 4 + j)

        oT0p0 = normalize(0, 0)
        for kc in range(KC):
            attn_chunk(0, 1, kc)
        prev = ([oT0p0, normalize(0, 1)],
                rs_dram.tile([NBLK * 128, D], BF16, tag="rs_in", name="rs_in"), 0)

        for blk in range(1, NBLK):
            oTb = []
            for p in range(2):
                for kc in range(KC):
                    attn_chunk(blk, p, kc)
                    # interleave the previous block's output projection and
                    # ReduceScatter into p0's kc loop so the exp heartbeat
                    # never stalls at block boundaries
                    if p == 0 and prev is not None:
                        if kc in (2, 5, 8, 11):
                            outproj_sq(prev[0], (kc - 2) // 3, prev[1])
                        elif kc == 14:
                            emit_rs(prev[2], prev[1])
                            prev = None
                oTb.append(normalize(blk, p))
            prev = (oTb,
                    rs_dram.tile([NBLK * 128, D], BF16, tag="rs_in", name="rs_in"),
                    blk)

        for sq in range(4):
            outproj_sq(prev[0], sq, prev[1])
        emit_rs(prev[2], prev[1])

    nc.compile()
    return nc


_NC = None


def kernel(x, W_qkv, b_qkv, W_out, b_out):
    global _NC
    if _NC is None:
        _NC = build_nc()

    cols = np.concatenate([np.arange(t * 1024, t * 1024 + 256) for t in range(3)])
    in_maps = []
    for c in range(8):
        b, g = c // 4, c % 4
        gcols = cols + g * 256
        in_maps.append({
            "x": np.ascontiguousarray(x[b]),
            "wqkv": np.ascontiguousarray(W_qkv[:, gcols]),
            "bqkv": np.ascontiguousarray(b_qkv[gcols]),
            "wout": np.ascontiguousarray(W_out[g * 256:(g + 1) * 256, :]),
            "bout": np.ascontiguousarray(b_out),
        })

    res = run_bass_kernel_spmd(_NC, in_maps, core_ids=list(range(8)))

    # core (b, g), local row r = blk*128 + j  <->  full row = blk*512 + g*128 + j
    out = np.empty((2, S, D), np.float32)
    for c in range(8):
        b, g = c // 4, c % 4
        r = res.results[c]["out"]
        for k in range(NBLK):
            out[b, k * BLK + g * 128: k * BLK + (g + 1) * 128, :] = \
                r[k * 128:(k + 1) * 128, :]
    return out
